# revision 14
# baseline (speedup 1.0000x reference)
"""DH-SFNN Trainium2 kernel (8 NeuronCores, data-parallel over batch).

Model: 2 dendritic LIF layers (K=4 branches, reset-by-subtraction) + leaky
readout integrator, T=250 steps, B=256, IN=700, H=256, O=20.

Fast path (per core, B_l=32), exploiting reset-by-subtraction soundness:
spike corrections are strictly subtractive, so if the no-spike layer-1
membrane trajectory m1^ satisfies max m1^ <= VTH there are exactly zero
layer-1 spikes. Layer 2 then sees only its bias trajectory (x-independent,
verified exactly on host), and the readout is a batch-independent constant
computed on host. The device therefore only needs to certify layer 1:

    c1 = x @ (16*W1).T (+bias row)     -- fp8 DoubleRow matmuls (2x128
                                          contraction rows per instr)
    d1 = per-feature 1-pole IIR over t -- DVE tensor_tensor_scan, 4 batch
                                          streams packed per instruction with
                                          zeroed-multiplier boundary columns
    D1 = sum_k g_k d1_k               -- PE matmul with g/16-weighted selector
    check max_t D1 <= VTH - 0.25      -- Act engine relu-accumulate; since
                                          m1^ is a running convex combination
                                          of D1, max m1^ <= max(0, max D1).

If the on-device flag fires, or the host-side layer-2 bias-trajectory check
fails, rerun with the general sequential-correction kernel (slow path).
"""
import sys

sys.path.insert(0, "/opt/trn_rl_repo")

import numpy as np
import ml_dtypes

import concourse.bass as bass
import concourse.mybir as mybir
import concourse.tile as tile
from concourse import bacc, bass_utils, bass_isa

F32 = mybir.dt.float32
BF16 = mybir.dt.bfloat16
FP8 = mybir.dt.float8e4
ALU = mybir.AluOpType
ACT = mybir.ActivationFunctionType
DR = mybir.MatmulPerfMode.DoubleRow

N_CORES = 8
B, T, IN, H, O, K = 256, 250, 700, 256, 20, 4
BL = B // N_CORES            # 32 batch per core
BBLK = 4                     # batches per scan slab
NBB = BL // BBLK             # 8 slabs
NSL = BBLK * T               # 1000 slab columns
IC = 6                       # 768 = 6*128 contraction rows (row 700 = bias)
NPR = IC // 2                # 3 DoubleRow pair chunks
NF = H * K                   # 1024 layer-1 branch features
NCF = NF // 128              # 8 feature chunks
VTH = 1.0
CHECK_MARGIN = 0.25          # device certifies max D <= VTH - margin
WSC = 16.0                   # power-of-2 prescale on W1 for fp8 range
# out-column splits of the 1000 slab columns, each within one PSUM bank
CSPLITS = [(0, 256), (256, 256), (512, 256), (768, 232)]
NN_SPLITS = [(0, 512), (512, 488)]


def _sig(v):
    return 1.0 / (1.0 + np.exp(-np.asarray(v, np.float64)))


def build_nc():
    nc = bacc.Bacc("TRN2", target_bir_lowering=False, debug=False,
                   num_devices=N_CORES)
    dt = nc.dram_tensor
    xq_d = dt("xq", [NPR, 128, 2, BL, T], FP8, kind="ExternalInput").ap()
    w1_d = dt("w1q", [NPR, 128, 2 * NF], FP8, kind="ExternalInput").ap()
    sel_d = dt("selm", [128, NCF * 32], BF16, kind="ExternalInput").ap()
    bsl_d = dt("bsl1", [NCF, 128, NSL], BF16, kind="ExternalInput").ap()
    outc_d = dt("outc", [O, BL], F32, kind="ExternalInput").ap()
    out_d = dt("out", [O, BL], F32, kind="ExternalOutput").ap()
    flag_d = dt("flag", [128, 1], F32, kind="ExternalOutput").ap()

    with tile.TileContext(nc) as tc:
        with tc.tile_pool(name="const", bufs=1) as cpool, \
             tc.tile_pool(name="xs", bufs=2) as xpool, \
             tc.tile_pool(name="ds", bufs=2) as dpool, \
             tc.tile_pool(name="small", bufs=1) as mpool:

            # ---- constants (issue order = SP issue order: PE deps first) ----
            w1sb = [cpool.tile([128, 2 * NF], FP8, name=f"w1sb{i}",
                               tag=f"w1_{i}") for i in range(NPR)]
            for i in range(NPR):
                nc.sync.dma_start(out=w1sb[i], in_=w1_d[i])
            bslsb = cpool.tile([128, NCF * NSL], BF16, name="bslsb")
            nc.sync.dma_start(out=bslsb[:, 0:NSL], in_=bsl_d[0])
            selsb = cpool.tile([128, NCF * 32], BF16, name="selsb")
            outcsb = cpool.tile([O, BL], F32, name="outcsb")
            biasc = mpool.tile([128, 1], F32, name="biasc")
            nc.vector.memset(biasc, -(VTH - CHECK_MARGIN))

            cnt = mpool.tile([128, 2 * NBB], F32, name="cnt")
            csum = mpool.tile([128, 1], F32, name="csum")
            junk = mpool.tile([128, NSL], BF16, name="junk")
            junk16 = mpool.tile([128, 2 * NBB], F32, name="junk16")

            with tc.tile_pool(name="psA", bufs=2, space="PSUM") as pspool, \
                 tc.tile_pool(name="psB", bufs=2, space="PSUM") as dppool:
                dss = {}

                def emit_x(bb):
                    xs = []
                    for pr in range(NPR):
                        t_ = xpool.tile([128, 2 * NSL], FP8,
                                        name=f"xs{bb}_{pr}", tag=f"xs{pr}")
                        nc.gpsimd.dma_start(
                            out=t_.rearrange("p (k b t) -> p k b t",
                                             k=2, b=BBLK),
                            in_=xq_d[pr][:, :, bb * BBLK:(bb + 1) * BBLK, :])
                        xs.append(t_.rearrange("p (k n) -> p k n", k=2))
                    return xs

                def emit_cmm_scan(bb, xs, cfs):
                    ds = dss[bb]
                    for cf in cfs:
                        ps = pspool.tile([128, 1024], F32,
                                         name=f"c{bb}_{cf}", tag="mm")
                        for n0, nw in CSPLITS:
                            for pr in range(NPR):
                                nc.tensor.matmul(
                                    ps[:, n0:n0 + nw],
                                    lhsT=w1sb[pr]
                                        .rearrange("p (k m) -> p k m", k=2)
                                        [:, :, cf * 128:(cf + 1) * 128],
                                    rhs=xs[pr][:, :, n0:n0 + nw],
                                    start=(pr == 0), stop=(pr == NPR - 1),
                                    perf_mode=DR)
                        nc.vector.tensor_tensor_scan(
                            out=ds[:, cf * NSL:(cf + 1) * NSL],
                            data0=bslsb[:, cf * NSL:(cf + 1) * NSL],
                            data1=ps[:, 0:NSL], initial=0.0,
                            op0=ALU.mult, op1=ALU.add)

                def emit_sel_check(bb, hh):
                    ds = dss[bb]
                    Dps = dppool.tile([128, 1024], F32,
                                      name=f"D{bb}_{hh}", tag="D")
                    for c4 in range(4):
                        cf = hh * 4 + c4
                        for n0, nw in NN_SPLITS:
                            nc.tensor.matmul(
                                Dps[c4 * 32:(c4 + 1) * 32, n0:n0 + nw],
                                lhsT=selsb[:, cf * 32:(cf + 1) * 32],
                                rhs=ds[:, cf * NSL + n0:cf * NSL + n0 + nw],
                                start=True, stop=True,
                                tile_position=(0, c4 * 32))
                    # spike certificate: relu(D - (VTH - margin)) summed
                    nc.scalar.activation(
                        out=junk, in_=Dps[:, 0:NSL], func=ACT.Relu,
                        bias=biasc, scale=1.0,
                        accum_out=cnt[:, bb * 2 + hh:bb * 2 + hh + 1])

                for bb in range(NBB):
                    dss[bb] = dpool.tile([128, NCF * NSL], BF16,
                                         name=f"ds{bb}", tag="ds")
                xs = emit_x(0)
                # remaining constants issue behind the critical first slab
                nc.sync.dma_start(out=selsb, in_=sel_d)
                for i in range(1, NCF):
                    nc.sync.dma_start(out=bslsb[:, i * NSL:(i + 1) * NSL],
                                      in_=bsl_d[i])
                nc.sync.dma_start(out=outcsb, in_=outc_d)
                nc.sync.dma_start(out=out_d, in_=outcsb)
                for bb in range(NBB):
                    emit_cmm_scan(bb, xs, range(0, 4))
                    if bb > 0:
                        emit_sel_check(bb - 1, 0)
                    emit_cmm_scan(bb, xs, range(4, NCF))
                    if bb + 1 < NBB:
                        xs = emit_x(bb + 1)
                    if bb > 0:
                        emit_sel_check(bb - 1, 1)
                    if bb == NBB - 1:
                        emit_sel_check(bb, 0)
                emit_sel_check(NBB - 1, 1)

            nc.scalar.activation(
                out=junk16, in_=cnt, func=ACT.Copy, bias=0.0, scale=1.0,
                accum_out=csum)
            nc.sync.dma_start(out=flag_d, in_=csum)

    nc.compile()
    return nc


# ---------------------------------------------------------------------------
# general fallback kernel (sequential spike-correction), used only when the
# no-spike certificate fails: identical to the reference recurrence.
# ---------------------------------------------------------------------------

def build_nc_slow():
    nc = bacc.Bacc("TRN2", target_bir_lowering=False, debug=False,
                   num_devices=N_CORES)
    dt = nc.dram_tensor
    xt_d = dt("xt", [IC * 128, BL, T], BF16, kind="ExternalInput").ap()
    w1_d = dt("w1p", [IC * 128, NF], BF16, kind="ExternalInput").ap()
    w2_d = dt("w2p", [H, NF], BF16, kind="ExternalInput").ap()
    wr_d = dt("wrt", [128, 2 * O], BF16, kind="ExternalInput").ap()
    m2b_d = dt("mh2b", [128, 2 * T], BF16, kind="ExternalInput").ap()
    bsl1_d = dt("bsl1", [NCF, 128, NSL], BF16, kind="ExternalInput").ap()
    bsl2_d = dt("bsl2", [NCF, 128, NSL], BF16, kind="ExternalInput").ap()
    asl_d = dt("asl", [128, 4 * NSL], BF16, kind="ExternalInput").ap()
    acol_d = dt("acol", [128, 4], F32, kind="ExternalInput").ap()
    sel_d = dt("selm", [128, 32], BF16, kind="ExternalInput").ap()
    ur_d = dt("ur", [O, T], F32, kind="ExternalInput").ap()
    bru_d = dt("bru", [O, 1], F32, kind="ExternalInput").ap()
    out_d = dt("out", [O, BL], F32, kind="ExternalOutput").ap()
    flag_d = dt("flag", [1, 2], F32, kind="ExternalOutput").ap()

    with tile.TileContext(nc) as tc:
        with tc.tile_pool(name="const", bufs=1) as cpool, \
             tc.tile_pool(name="state", bufs=1) as spool, \
             tc.tile_pool(name="bsl", bufs=1) as bpool, \
             tc.tile_pool(name="xs", bufs=2) as xpool, \
             tc.tile_pool(name="ds", bufs=2) as dpool, \
             tc.tile_pool(name="small", bufs=1) as mpool:

            w1sb = [cpool.tile([128, NF], BF16, name=f"w1sb{i}", tag=f"w1_{i}")
                    for i in range(IC)]
            for i in range(IC):
                nc.sync.dma_start(out=w1sb[i], in_=w1_d[i * 128:(i + 1) * 128, :])
            w2sb = [cpool.tile([128, NF], BF16, name=f"w2sb{i}", tag=f"w2_{i}")
                    for i in range(2)]
            for i in range(2):
                nc.sync.dma_start(out=w2sb[i], in_=w2_d[i * 128:(i + 1) * 128, :])
            wrsb = cpool.tile([128, 2 * O], BF16, name="wrsb")
            nc.sync.dma_start(out=wrsb, in_=wr_d)
            m2bsb = cpool.tile([128, 2 * T], BF16, name="m2bsb")
            nc.sync.dma_start(out=m2bsb, in_=m2b_d)
            aslsb = cpool.tile([128, 4 * NSL], BF16, name="aslsb")
            nc.sync.dma_start(out=aslsb, in_=asl_d)
            acolsb = cpool.tile([128, 4], F32, name="acolsb")
            nc.sync.dma_start(out=acolsb, in_=acol_d)
            selsb = cpool.tile([128, 32], BF16, name="selsb")
            nc.sync.dma_start(out=selsb, in_=sel_d)
            ursb = cpool.tile([O, T], F32, name="ursb")
            nc.sync.dma_start(out=ursb, in_=ur_d)
            brusb = cpool.tile([O, 1], F32, name="brusb")
            nc.sync.dma_start(out=brusb, in_=bru_d)

            mhat = spool.tile([128, 2 * NBB * NSL], BF16, name="mhat")
            sfull = spool.tile([128, 2 * NBB * NSL], BF16, name="sfull")
            q = mpool.tile([128, 64], BF16, name="q")
            cnt = mpool.tile([128, 4], F32, name="cnt")
            csum = mpool.tile([128, 2], F32, name="csum")
            par = mpool.tile([128, 2], F32, name="par")
            acc = mpool.tile([O, BL], F32, name="acc")
            accb = mpool.tile([O, BL], F32, name="accb")
            zjunk = mpool.tile([O, T], F32, name="zjunk")

            mh_v = mhat.rearrange("p (hh b t) -> p hh b t", hh=2, b=BL, t=T)
            sf_v = sfull.rearrange("p (hh b t) -> p hh b t", hh=2, b=BL, t=T)
            q_v = q.rearrange("p (hh b) -> p hh b", hh=2)

            with tc.tile_pool(name="psA", bufs=2, space="PSUM") as pspool:

                def layer(L, bsl_d, rhs_mm):
                    bslsb = bpool.tile([128, NCF * NSL], BF16, name=f"bslsb{L}",
                                       tag="bsl")
                    for cf in range(NCF):
                        nc.sync.dma_start(out=bslsb[:, cf * NSL:(cf + 1) * NSL],
                                          in_=bsl_d[cf])
                    aoff = (L - 1) * 2 * NSL
                    for bb in range(NBB):
                        ds = dpool.tile([128, NCF * NSL], BF16,
                                        name=f"ds{L}_{bb}", tag="ds")
                        for cf in range(NCF):
                            ps = pspool.tile([128, NSL], F32,
                                             name=f"c{L}_{bb}_{cf}", tag="mm")
                            for nn in range(2):
                                rhs_mm(ps, bb, cf, nn)
                            nc.vector.tensor_tensor_scan(
                                out=ds[:, cf * NSL:(cf + 1) * NSL],
                                data0=bslsb[:, cf * NSL:(cf + 1) * NSL],
                                data1=ps,
                                initial=0.0, op0=ALU.mult, op1=ALU.add)
                        for hh in range(2):
                            Dps = pspool.tile([128, 1024], F32,
                                              name=f"D{L}_{bb}_{hh}", tag="D")
                            for c4 in range(4):
                                o4 = (hh * 4 + c4) * NSL
                                for n0, nw in NN_SPLITS:
                                    nc.tensor.matmul(
                                        Dps[c4 * 32:(c4 + 1) * 32,
                                            n0:n0 + nw],
                                        lhsT=selsb,
                                        rhs=ds[:, o4 + n0:o4 + n0 + nw],
                                        start=True, stop=True,
                                        tile_position=(0, c4 * 32))
                            nc.vector.tensor_tensor_scan(
                                out=mhat[:, hh * 8000 + bb * NSL:
                                         hh * 8000 + (bb + 1) * NSL],
                                data0=aslsb[:, aoff + hh * NSL:
                                            aoff + (hh + 1) * NSL],
                                data1=Dps[:, 0:NSL], initial=0.0,
                                op0=ALU.mult, op1=ALU.add)

                def spike_phase(L):
                    nc.gpsimd.memset(sfull, 0.0)
                    junk = dpool.tile([128, NCF * NSL], BF16,
                                      name=f"junk{L}", tag="ds")
                    for hh in range(2):
                        nc.vector.tensor_scalar(
                            out=junk[:, 0:8000],
                            in0=mhat[:, hh * 8000:(hh + 1) * 8000],
                            scalar1=float(VTH), scalar2=None, op0=ALU.is_gt,
                            op1=ALU.add,
                            accum_out=cnt[:, (L - 1) * 2 + hh:(L - 1) * 2 + hh + 1])
                    nc.vector.tensor_add(
                        out=csum[:, L - 1:L],
                        in0=cnt[:, (L - 1) * 2:(L - 1) * 2 + 1],
                        in1=cnt[:, (L - 1) * 2 + 1:(L - 1) * 2 + 2])
                    nc.gpsimd.partition_all_reduce(
                        par[:, L - 1:L], csum[:, L - 1:L], channels=128,
                        reduce_op=bass_isa.ReduceOp.add)
                    nc.vector.memset(q, 0.0)
                    for t in range(T):
                        nc.vector.scalar_tensor_tensor(
                            out=sf_v[:, :, :, t], in0=mh_v[:, :, :, t],
                            scalar=float(VTH), op0=ALU.subtract,
                            in1=q_v, op1=ALU.is_gt)
                        for hh in range(2):
                            nc.vector.scalar_tensor_tensor(
                                out=q[:, hh * 32:(hh + 1) * 32],
                                in0=q[:, hh * 32:(hh + 1) * 32],
                                scalar=acolsb[:, (L - 1) * 2 + hh:
                                              (L - 1) * 2 + hh + 1],
                                op0=ALU.mult,
                                in1=sf_v[:, hh, :, t], op1=ALU.add)

                xs = {}

                def mm1(ps, bb, cf, nn):
                    n0, nw = NN_SPLITS[nn]
                    if cf == 0 and nn == 0:
                        for i in range(IC):
                            t_ = xpool.tile([128, NSL], BF16,
                                            name=f"xs{bb}_{i}", tag=f"xs{i}")
                            nc.sync.dma_start(
                                out=t_.rearrange("p (b t) -> p b t", b=BBLK),
                                in_=xt_d[i * 128:(i + 1) * 128,
                                         bb * BBLK:(bb + 1) * BBLK, :])
                            xs[i] = t_
                    for i in range(IC):
                        nc.tensor.matmul(
                            ps[:, n0:n0 + nw],
                            lhsT=w1sb[i][:, cf * 128:(cf + 1) * 128],
                            rhs=xs[i][:, n0:n0 + nw],
                            start=(i == 0), stop=(i == IC - 1))

                layer(1, bsl1_d, mm1)
                spike_phase(1)

                def mm2(ps, bb, cf, nn):
                    n0, nw = NN_SPLITS[nn]
                    for hh in range(2):
                        nc.tensor.matmul(
                            ps[:, n0:n0 + nw],
                            lhsT=w2sb[hh][:, cf * 128:(cf + 1) * 128],
                            rhs=sfull[:, hh * 8000 + bb * NSL + n0:
                                      hh * 8000 + bb * NSL + n0 + nw],
                            start=(hh == 0), stop=(hh == 1))

                layer(2, bsl2_d, mm2)
                nc.vector.tensor_add(
                    out=mh_v, in0=mh_v,
                    in1=m2bsb.rearrange("p (hh t) -> p hh t", hh=2)
                        .unsqueeze(2).broadcast_to((128, 2, BL, T)))
                spike_phase(2)

            with tc.tile_pool(name="psB", bufs=2, space="PSUM") as zpool:
                for bb in range(NBB):
                    for nn in range(2):
                        zps = zpool.tile([O, 500], F32, name=f"z{bb}_{nn}",
                                         tag="z")
                        for hh in range(2):
                            nc.tensor.matmul(
                                zps,
                                lhsT=wrsb[:, hh * O:(hh + 1) * O],
                                rhs=sfull[:, hh * 8000 + bb * NSL + nn * 500:
                                          hh * 8000 + bb * NSL + (nn + 1) * 500],
                                start=(hh == 0), stop=(hh == 1))
                        for b2 in range(2):
                            b = bb * BBLK + nn * 2 + b2
                            nc.vector.scalar_tensor_tensor(
                                out=zjunk, in0=zps[:, b2 * T:(b2 + 1) * T],
                                scalar=1.0, op0=ALU.mult,
                                in1=ursb, op1=ALU.mult,
                                accum_out=acc[:, b:b + 1])
                nc.vector.tensor_scalar(
                    out=accb, in0=acc, scalar1=brusb[:, 0:1], scalar2=None,
                    op0=ALU.add)
                nc.sync.dma_start(out=out_d, in_=accb)
                nc.sync.dma_start(out=flag_d, in_=par[0:1, 0:2])

    nc.compile()
    return nc


_NC_CACHE = {}


def get_nc():
    if "fast" not in _NC_CACHE:
        _NC_CACHE["fast"] = build_nc()
    return _NC_CACHE["fast"]


def get_nc_slow():
    if "slow" not in _NC_CACHE:
        _NC_CACHE["slow"] = build_nc_slow()
    return _NC_CACHE["slow"]


def prep_inputs(x, W1, b1, tau_n1, tau_m1, W2, b2, tau_n2, tau_m2,
                Wr, br, tau_mr, warmup):
    """Host-side: per-core input dicts for the fast bass kernel, plus the
    host-verified layer-2/readout constants. Returns (in_maps, fast_ok)."""
    w = int(np.asarray(warmup))
    beta1 = _sig(tau_n1).reshape(NF)          # [H,K], j = h*4+k order
    alpha1 = _sig(tau_m1)                     # [H]
    beta2 = _sig(tau_n2).reshape(NF)
    alpha2 = _sig(tau_m2)
    alphar = _sig(tau_mr)                     # [O]

    g1 = (1.0 - beta1) * np.repeat(1.0 - alpha1, K)

    # fp8 weights, prescaled by WSC; row 700 = bias, rows 701.. = 0
    w1t = np.zeros((IC * 128, NF), np.float64)
    w1t[:IN] = np.asarray(W1, np.float64).T * WSC
    w1t[IN] = np.asarray(b1, np.float64) * WSC
    w1q = np.empty((NPR, 128, 2 * NF), ml_dtypes.float8_e4m3)
    for pr in range(NPR):
        w1q[pr, :, :NF] = w1t[2 * pr * 128:(2 * pr + 1) * 128]
        w1q[pr, :, NF:] = w1t[(2 * pr + 1) * 128:(2 * pr + 2) * 128]

    # selector: g/WSC weights, [128, 32] blocks per feature chunk, packed
    selm = np.zeros((128, NCF * 32), ml_dtypes.bfloat16)
    for cf in range(NCF):
        j = cf * 128 + np.arange(128)
        selm[np.arange(128), cf * 32 + np.arange(128) // 4] = g1[j] / WSC

    def bslab(beta):
        s = np.tile(beta.reshape(NCF, 128, 1).astype(ml_dtypes.bfloat16),
                    (1, 1, NSL))
        s.reshape(NCF, 128, BBLK, T)[:, :, :, 0] = 0.0
        return s

    bsl1 = bslab(beta1)

    # host-exact layer-2 bias trajectory (valid when layer 1 has no spikes)
    b2g = np.asarray(b2, np.float64) * (1.0 - beta2)
    dtraj = np.zeros(NF)
    mtraj = np.zeros(H)
    m2max = -np.inf
    for _ in range(T):
        dtraj = beta2 * dtraj + b2g
        mtraj = alpha2 * mtraj + (1.0 - alpha2) * dtraj.reshape(H, K).sum(-1)
        m2max = max(m2max, mtraj.max())
    fast_ok = bool(m2max <= VTH - 0.05)

    # host-exact readout constant (valid when layer 2 has no spikes)
    mr = np.zeros(O)
    accr = np.zeros(O)
    for t_ in range(T):
        mr = mr * alphar + (1.0 - alphar) * np.asarray(br, np.float64)
        if t_ >= w:
            accr += mr
    outc = np.tile((accr / (T - w)).astype(np.float32)[:, None], (1, BL))

    xq_full = np.zeros((IC * 128, B, T), ml_dtypes.float8_e4m3)
    xq_full[:IN] = np.asarray(x).transpose(2, 0, 1)
    xq_full[IN] = 1.0
    # pair-interleaved: [NPR, 128, 2, B, T]
    xq_full = np.ascontiguousarray(
        xq_full.reshape(NPR, 2, 128, B, T).transpose(0, 2, 1, 3, 4))

    shared = dict(w1q=w1q, selm=selm, bsl1=bsl1, outc=outc)
    in_maps = []
    for c in range(N_CORES):
        m = dict(shared)
        m["xq"] = np.ascontiguousarray(
            xq_full[:, :, :, c * BL:(c + 1) * BL, :])
        in_maps.append(m)
    return in_maps, fast_ok


def prep_inputs_slow(x, W1, b1, tau_n1, tau_m1, W2, b2, tau_n2, tau_m2,
                     Wr, br, tau_mr, warmup):
    """Host-side prep for the general fallback kernel."""
    w = int(np.asarray(warmup))
    beta1 = _sig(tau_n1).reshape(NF)
    alpha1 = _sig(tau_m1)
    beta2 = _sig(tau_n2).reshape(NF)
    alpha2 = _sig(tau_m2)
    alphar = _sig(tau_mr)

    g1 = (1.0 - beta1) * np.repeat(1.0 - alpha1, K)
    g2 = (1.0 - beta2) * np.repeat(1.0 - alpha2, K)

    w1p = np.zeros((IC * 128, NF), np.float64)
    w1p[:IN] = np.asarray(W1, np.float64).T * g1
    w1p[IN] = np.asarray(b1, np.float64) * g1
    w1p = w1p.astype(ml_dtypes.bfloat16)

    w2p = (np.asarray(W2, np.float64).T * g2).astype(ml_dtypes.bfloat16)
    b2g = np.asarray(b2, np.float64) * g2
    dtraj = np.zeros(NF)
    mh2b = np.zeros((H, T))
    mtraj = np.zeros(H)
    for t_ in range(T):
        dtraj = _sig(tau_n2).reshape(NF) * dtraj + b2g
        mtraj = _sig(tau_m2) * mtraj + dtraj.reshape(H, K).sum(-1)
        mh2b[:, t_] = mtraj
    mh2b_dev = np.zeros((128, 2 * T), np.float64)
    mh2b_dev[:, :T] = mh2b[:128]
    mh2b_dev[:, T:] = mh2b[128:]
    mh2b_dev = mh2b_dev.astype(ml_dtypes.bfloat16)

    wrt = np.zeros((128, 2 * O), np.float64)
    wrt[:, :O] = np.asarray(Wr, np.float64).T[:128]
    wrt[:, O:] = np.asarray(Wr, np.float64).T[128:]
    wrt = wrt.astype(ml_dtypes.bfloat16)

    def bslab(beta):
        s = np.tile(beta.reshape(NCF, 128, 1).astype(ml_dtypes.bfloat16),
                    (1, 1, NSL))
        s.reshape(NCF, 128, BBLK, T)[:, :, :, 0] = 0.0
        return s

    bsl1 = bslab(beta1)
    bsl2 = bslab(beta2)

    def aslab(alpha):
        a2 = alpha.reshape(2, 128).astype(ml_dtypes.bfloat16)
        s = np.tile(a2[:, :, None], (1, 1, NSL))
        s.reshape(2, 128, BBLK, T)[:, :, :, 0] = 0.0
        return s

    asl = np.concatenate([aslab(alpha1), aslab(alpha2)], axis=0)
    asl = asl.transpose(1, 0, 2).reshape(128, 4 * NSL).copy()

    acol = np.stack([alpha1[:128], alpha1[128:], alpha2[:128], alpha2[128:]],
                    axis=1).astype(np.float32)

    selm = np.zeros((128, 32), ml_dtypes.bfloat16)
    selm[np.arange(128), np.arange(128) // 4] = 1.0

    tt = np.arange(T, dtype=np.float64)[:, None]
    ar = alphar[None, :]
    u = ar ** np.maximum(0, w - tt) - ar ** (T - tt)
    ur = (u.T / (T - w)).astype(np.float32)
    bru = (np.asarray(br, np.float64) * u.sum(0) / (T - w)) \
        .astype(np.float32)[:, None]

    xt_full = np.zeros((IC * 128, B, T), ml_dtypes.bfloat16)
    xt_full[:IN] = np.asarray(x).transpose(2, 0, 1)
    xt_full[IN] = 1.0

    shared = dict(w1p=w1p, w2p=w2p, mh2b=mh2b_dev, wrt=wrt,
                  bsl1=bsl1, bsl2=bsl2, asl=asl, acol=acol, selm=selm,
                  ur=ur, bru=bru)
    in_maps = []
    for c in range(N_CORES):
        m = dict(shared)
        m["xt"] = np.ascontiguousarray(xt_full[:, c * BL:(c + 1) * BL, :])
        in_maps.append(m)
    return in_maps


def _run_slow(**inputs):
    in_maps = prep_inputs_slow(**inputs)
    res = bass_utils.run_bass_kernel_spmd(
        get_nc_slow(), in_maps, core_ids=list(range(N_CORES)))
    out = np.empty((B, O), np.float32)
    for c in range(N_CORES):
        out[c * BL:(c + 1) * BL] = res.results[c]["out"].T
    return out


def kernel(**inputs):
    in_maps, fast_ok = prep_inputs(**inputs)
    if not fast_ok:
        return _run_slow(**inputs)
    res = bass_utils.run_bass_kernel_spmd(
        get_nc(), in_maps, core_ids=list(range(N_CORES)))
    if any(r["flag"].sum() > 0 for r in res.results):
        # certificate failed: spikes may exist, use the general kernel
        return _run_slow(**inputs)
    out = np.empty((B, O), np.float32)
    for c in range(N_CORES):
        out[c * BL:(c + 1) * BL] = res.results[c]["out"].T
    return out
